# revision 4
# baseline (speedup 1.0000x reference)
"""Trainium2 Bass kernel for the layered-circuit WMC problem.

Computation (see reference): vals = [weights | neg_weights]  # [B, 8192]
12 alternating AND(prod)/OR(sum) layers, each gathering 2 children per node
from the previous layer's 8192 node values, then a final sum over nodes.

Sharding: data-parallel over batch; 8 cores x 128 batch rows each.

Device design (per core) — all-SBUF, no HBM round trips:
- Node values live in SBUF batch-major: V[128 batch partitions, 8192 nodes]
  fp32 (32KB/partition), double-buffered.
- Per layer: GPSIMD `ap_gather` pulls 16384 child values along the free axis
  (same index list for every partition; idx interleaved c0,c1) in 4 chunks of
  4096; the Vector engine combines adjacent pairs (stride-2 APs) with
  mult (AND) / add (OR), writing the next layer's V. Chunks pipeline
  GPSIMD-gather against DVE-combine.
- The last OR layer + root sum collapse into a count-weighted reduction
  (host precomputes how often each layer-10 node appears in layer 11),
  done as chunked tensor_tensor_reduce against a replicated count row.

The compiled NEFF is input-independent (indices are runtime data) and is
cached across calls.
"""

import numpy as np

N_LAYERS = 12
DEV_LAYERS = 11  # layers 0..10 on device; layer 11 + root folded into counts
WIDTH = 8192
N_VARS = 4096
BATCH = 1024
N_CORES = 8
PB = BATCH // N_CORES  # 128 batch rows per core
NC = 4  # gather chunks per layer
CHUNK = 2 * WIDTH // NC  # 4096 idxs per chunk
HNODE = CHUNK // 2  # 2048 nodes produced per chunk
IDXP = CHUNK // 16  # idx int16 per partition per chunk
LIDX = 2 * WIDTH // 16  # 1024 idx int16 per partition per layer

_CACHE = {}


def _build_nc():
    import concourse.bacc as bacc
    import concourse.mybir as mybir
    from concourse import library_config

    f32 = mybir.dt.float32
    i16 = mybir.dt.int16

    nc = bacc.Bacc("TRN2", target_bir_lowering=False, debug=False)

    v0d = nc.dram_tensor("v0", [PB, WIDTH], f32, kind="ExternalInput")
    idxd = nc.dram_tensor("idxs", [PB, DEV_LAYERS * LIDX], i16, kind="ExternalInput")
    cntd = nc.dram_tensor("cnt", [PB, WIDTH], f32, kind="ExternalInput")
    outd = nc.dram_tensor("out", [PB, 1], f32, kind="ExternalOutput")

    with (
        nc.sbuf_tensor("va", [PB, WIDTH], f32) as va,
        nc.sbuf_tensor("vb", [PB, WIDTH], f32) as vb,
        nc.sbuf_tensor("g0", [PB, CHUNK], f32) as g0,
        nc.sbuf_tensor("g1", [PB, CHUNK], f32) as g1,
        nc.sbuf_tensor("idx_sb", [PB, DEV_LAYERS * LIDX], i16) as idx_sb,
        nc.sbuf_tensor("cnt_sb", [PB, WIDTH], f32) as cnt_sb,
        nc.sbuf_tensor("res", [PB, 1], f32) as res,
        nc.semaphore("vsem") as vsem,
        nc.semaphore("isem") as isem,
        nc.semaphore("nsem") as nsem,
        nc.semaphore("gsem") as gsem,
        nc.semaphore("csem") as csem,
        nc.semaphore("psem") as psem,
        nc.semaphore("osem") as osem,
        nc.Block() as block,
    ):
        vbuf = [va, vb]
        gbuf = [g0, g1]

        @block.gpsimd
        def _(g):
            g.load_library(library_config.ap_gather)
            g.wait_ge(vsem, 16)  # leaves loaded
            g.wait_ge(isem, 16)  # idx lists loaded
            for l in range(DEV_LAYERS):
                for c in range(NC):
                    k = l * NC + c
                    if c < 2:
                        if l > 0:
                            g.wait_ge(csem, l * NC)  # V_l fully combined
                    else:
                        g.wait_ge(csem, k - 1)  # g[k%2] free
                    g.ap_gather(
                        gbuf[k % 2][:],
                        vbuf[l % 2][:],
                        idx_sb[:, l * LIDX + c * IDXP : l * LIDX + (c + 1) * IDXP],
                        PB,
                        WIDTH,
                        1,
                        CHUNK,
                    ).then_inc(gsem, 1)

        @block.vector
        def _(v):
            for l in range(DEV_LAYERS):
                op = mybir.AluOpType.mult if l % 2 == 0 else mybir.AluOpType.add
                vnext = vbuf[(l + 1) % 2]
                for c in range(NC):
                    k = l * NC + c
                    v.wait_ge(gsem, k + 1)
                    gp = gbuf[k % 2][:].rearrange("p (j two) -> p two j", two=2)
                    v.tensor_tensor(
                        out=vnext[:, c * HNODE : (c + 1) * HNODE],
                        in0=gp[:, 0, :],
                        in1=gp[:, 1, :],
                        op=op,
                    ).then_inc(csem, 1)
            # WMC = sum(V10 * cnt) per batch row; V10 is in vb, va is dead.
            v.wait_ge(nsem, 16)  # counts loaded
            v.wait_ge(csem, DEV_LAYERS * NC)  # drain last combine's write
            v.tensor_tensor(
                out=va[:],
                in0=vbuf[DEV_LAYERS % 2][:],
                in1=cnt_sb[:],
                op=mybir.AluOpType.mult,
            ).then_inc(csem, 1)
            v.wait_ge(csem, DEV_LAYERS * NC + 1)  # drain weighted product
            v.tensor_reduce(
                out=res[:],
                in_=va[:],
                axis=mybir.AxisListType.X,
                op=mybir.AluOpType.add,
            ).then_inc(psem, 1)

        @block.sync
        def _(s):
            s.dma_start(va[:], v0d[:]).then_inc(vsem, 16)
            s.dma_start(idx_sb[:], idxd[:]).then_inc(isem, 16)
            s.dma_start(cnt_sb[:], cntd[:]).then_inc(nsem, 16)
            s.wait_ge(psem, 1)
            s.dma_start(outd[:], res[:]).then_inc(osem, 16)
            s.wait_ge(osem, 16)

    nc.compile()
    return nc


def _get_nc():
    if "nc" not in _CACHE:
        _CACHE["nc"] = _build_nc()
    return _CACHE["nc"]


def _wrap_idx(idx_list):
    """int16 wrapped layout: list position i -> partition i%16 (replicated
    across the 8 Q7 cores), int16 free position i//16."""
    return np.tile(idx_list.reshape(-1, 16).T, (8, 1)).astype(np.int16)


def _prep_inputs(weights, neg_weights, children):
    w = np.asarray(weights, np.float32)
    nw = np.asarray(neg_weights, np.float32)
    ch = np.asarray(children, np.int64)

    leaves = np.concatenate([w, nw], axis=1)  # [1024, 8192]

    # interleaved per-layer gather lists: idx[2j] = c0[j], idx[2j+1] = c1[j]
    idx_blocks = []
    for l in range(DEV_LAYERS):
        inter = np.empty(2 * WIDTH, dtype=np.int16)
        inter[0::2] = ch[l, :, 0]
        inter[1::2] = ch[l, :, 1]
        idx_blocks.append(_wrap_idx(inter))
    idx_arr = np.ascontiguousarray(np.concatenate(idx_blocks, axis=1))

    # layer-11 counts over layer-10 outputs, replicated across partitions
    count11 = np.bincount(ch[11].ravel(), minlength=WIDTH).astype(np.float32)
    cnt_rep = np.ascontiguousarray(np.broadcast_to(count11, (PB, WIDTH)))

    in_maps = []
    for c in range(N_CORES):
        v0 = np.ascontiguousarray(leaves[c * PB : (c + 1) * PB])  # [128, 8192]
        in_maps.append({"v0": v0, "idxs": idx_arr, "cnt": cnt_rep})
    return in_maps


def run(weights, neg_weights, children, trace=False):
    from concourse.bass_utils import run_bass_kernel_spmd

    nc = _get_nc()
    in_maps = _prep_inputs(weights, neg_weights, children)
    br = run_bass_kernel_spmd(nc, in_maps, list(range(N_CORES)), trace=trace)
    out = np.concatenate([r["out"][:, 0] for r in br.results]).astype(np.float32)
    return out, br


def kernel(weights, neg_weights, children):
    out, _ = run(weights, neg_weights, children)
    return out
